# revision 18
# baseline (speedup 1.0000x reference)
"""Trainium2 Bass kernel for nn_Bidirectional_Interaction_Type_Attention.

Contract: kernel(**inputs) takes FULL unsharded inputs (numpy, as produced by
setup_inputs) and returns the FULL output tuple (pred [16,1] f32,
final_feat_norm [16,263] f32).

Sharding: data-parallel over B across 8 NeuronCores (2 samples/core). All
reductions are per-sample so no collectives are needed; small MLP params are
replicated to every core.

Per-core math (Bc=2 samples, R=1024 in 8 tiles of 128 rows):
  E  = exp(logits)              (bf16, ACT)
  EW = E * w_k                  (per-k tensor_scalar, DVE 4x)
  den/num = k-trees over E/EW   (bf16 TT adds, DVE 2x)
  S  = num * (1/den) * pm * fm
  softmax over F computed in S^T space (DMA-transpose), softmax over R in
  natural space; attention contexts + enriched embeddings via PE matmuls;
  type_strength via PE column-sum of WP = EW * (1/den); pooling via PE
  matvecs; MLP head on device.
"""

import sys
import numpy as np

for _p in ("/opt/trn_rl_repo",):
    if _p not in sys.path:
        sys.path.insert(0, _p)

import ml_dtypes

BF16 = ml_dtypes.bfloat16

B, R, F, D, K, H = 16, 1024, 128, 128, 7, 512
NCORES = 8
BC = B // NCORES          # samples per core
NT = R // 128             # R tiles per sample
FEAT = 2 * D + K          # 263
NEG = -1e9


def _softplus64(x):
    x = np.asarray(x, np.float64)
    return np.log1p(np.exp(-np.abs(x))) + np.maximum(x, 0.0)


def build_program(w, b3):
    """Build the per-core Bass program. w: softplus(type_weight) [K] float.
    b3: final bias scalar (baked as immediate)."""
    import concourse.bass as bass
    import concourse.mybir as mybir
    from concourse import bacc, tile

    f32 = mybir.dt.float32
    bf16 = mybir.dt.bfloat16
    Exp = mybir.ActivationFunctionType.Exp
    Relu = mybir.ActivationFunctionType.Relu
    Ln = mybir.ActivationFunctionType.Ln
    Square = mybir.ActivationFunctionType.Square
    mult = mybir.AluOpType.mult
    add = mybir.AluOpType.add

    nc = bacc.Bacc(None)

    # ---- DRAM I/O ----
    lg = nc.declare_dram_parameter("lg", [BC, R, K + 1, F], bf16, isOutput=False)
    le = nc.declare_dram_parameter("le", [BC, F, D], bf16, isOutput=False)
    fmr = nc.declare_dram_parameter("fmr", [BC, F], f32, isOutput=False)       # fg_mask rows
    fmnc = nc.declare_dram_parameter("fmnc", [BC, F], f32, isOutput=False)     # (1-fg)*NEG as col source
    pmT = nc.declare_dram_parameter("pmT", [BC, 128, NT], f32, isOutput=False)   # pm[s] cols per tile
    pmnT = nc.declare_dram_parameter("pmnT", [BC, 128, NT], f32, isOutput=False)  # (1-pm)*NEG cols
    wrT = nc.declare_dram_parameter("wrT", [D, D], bf16, isOutput=False)   # Wr^T
    wfT = nc.declare_dram_parameter("wfT", [D, D], bf16, isOutput=False)   # Wf^T
    brd = nc.declare_dram_parameter("brd", [128, D], bf16, isOutput=False)
    bfd = nc.declare_dram_parameter("bfd", [128, D], bf16, isOutput=False)
    fmBd = nc.declare_dram_parameter("fmBd", [BC, 128, F], bf16, isOutput=False)
    wrd = nc.declare_dram_parameter("wrd", [128, K, F], bf16, isOutput=False)
    w1d = nc.declare_dram_parameter("w1d", [3, 128, H], f32, isOutput=False)   # W1^T padded 263->384
    b1d = nc.declare_dram_parameter("b1d", [128, 4], f32, isOutput=False)       # b1 col-chunks
    w2d = nc.declare_dram_parameter("w2d", [4, 128, H // 2], f32, isOutput=False)  # W2^T chunks
    b2d = nc.declare_dram_parameter("b2d", [128, 2], f32, isOutput=False)
    w3d = nc.declare_dram_parameter("w3d", [2, 128, 1], f32, isOutput=False)   # W3^T chunks
    ident = nc.declare_dram_parameter("ident", [128, 128], bf16, isOutput=False)

    predT = nc.declare_dram_parameter("predT", [1, BC], f32, isOutput=True)
    featn = nc.declare_dram_parameter("featn", [BC, FEAT], f32, isOutput=True)

    with tile.TileContext(nc) as tc:
        with (
            tc.tile_pool(name="const", bufs=1) as cpool,
            tc.tile_pool(name="stream", bufs=3) as sp,
            tc.tile_pool(name="small", bufs=3) as sm,
            tc.tile_pool(name="keep", bufs=1, space="SBUF") as kp,
            tc.tile_pool(name="samp", bufs=1) as smp,
            tc.tile_pool(name="ps_persist", bufs=1, space="PSUM") as ppp,
            tc.tile_pool(name="ps_stream", bufs=1, space="PSUM") as pss,
        ):
            # ---- constants ----
            ones_col = cpool.tile([128, 1], bf16, tag="ones")
            nc.vector.memset(ones_col[:], 1.0)
            id_t = cpool.tile([128, 128], bf16, tag="ident")
            nc.sync.dma_start(out=id_t[:], in_=ident[:])
            wr_t = cpool.tile([D, D], bf16, tag="wrT")
            nc.sync.dma_start(out=wr_t[:], in_=wrT[:])
            wf_t = cpool.tile([D, D], bf16, tag="wfT")
            nc.sync.dma_start(out=wf_t[:], in_=wfT[:])
            brB = cpool.tile([128, D], bf16, tag="brB")
            nc.sync.dma_start(out=brB[:], in_=brd[:])
            bfB = cpool.tile([128, D], bf16, tag="bfB")
            nc.sync.dma_start(out=bfB[:], in_=bfd[:])
            id1f = cpool.tile([1, 1], f32, tag="id1f")
            nc.vector.memset(id1f[:], 1.0)
            wrep = cpool.tile([128, K, F], bf16, tag="wrep")
            nc.sync.dma_start(out=wrep[:], in_=wrd[:])
            w1_t = [cpool.tile([128, H], f32, tag=f"w1_{c}", name=f"w1_{c}") for c in range(3)]
            for c in range(3):
                nc.sync.dma_start(out=w1_t[c][:], in_=w1d[c])
            b1_t = cpool.tile([128, 4], f32, tag="b1")
            nc.sync.dma_start(out=b1_t[:], in_=b1d[:])
            w2_t = [cpool.tile([128, H // 2], f32, tag=f"w2_{j}", name=f"w2_{j}") for j in range(4)]
            for j in range(4):
                nc.sync.dma_start(out=w2_t[j][:], in_=w2d[j])
            b2_t = cpool.tile([128, 2], f32, tag="b2")
            nc.sync.dma_start(out=b2_t[:], in_=b2d[:])
            w3_t = [cpool.tile([128, 1], f32, tag=f"w3_{j}", name=f"w3_{j}") for j in range(2)]
            for j in range(2):
                nc.sync.dma_start(out=w3_t[j][:], in_=w3d[j])

            # ---- per-sample masks/ligand ----
            masks = {}
            for s in range(BC):
                fm_row = smp.tile([1, F], f32, tag=f"fmrow{s}", name=f"fmrow{s}")
                nc.sync.dma_start(out=fm_row[:], in_=fmr[s].unsqueeze(0))
                fm_rowb = smp.tile([1, F], bf16, tag=f"fmrowb{s}", name=f"fmrowb{s}")
                nc.vector.tensor_copy(fm_rowb[:], fm_row[:])
                fmn_col = smp.tile([128, 1], f32, tag=f"fmncol{s}", name=f"fmncol{s}")
                nc.sync.dma_start(out=fmn_col[:], in_=fmnc[s].unsqueeze(1))
                pm_cols = smp.tile([128, NT], f32, tag=f"pmcols{s}", name=f"pmcols{s}")
                nc.sync.dma_start(out=pm_cols[:], in_=pmT[s])
                pmn_cols = smp.tile([128, NT], f32, tag=f"pmncols{s}", name=f"pmncols{s}")
                nc.sync.dma_start(out=pmn_cols[:], in_=pmnT[s])
                lig_t = smp.tile([F, D], bf16, tag=f"lig{s}", name=f"lig{s}")
                nc.sync.dma_start(out=lig_t[:], in_=le[s])
                fmB = smp.tile([128, F], bf16, tag=f"fmB{s}", name=f"fmB{s}")
                nc.sync.dma_start(out=fmB[:], in_=fmBd[s])
                masks[s] = (fm_rowb, fmn_col, pm_cols, pmn_cols, lig_t, fmB)

            featT = [sm.tile([128, BC], f32, tag=f"ft{c}", name=f"ft{c}", bufs=1)
                     for c in range(3)]
            norm_defer = []
            nc.vector.memset(featT[2][:], 0.0)

            for s in range(BC):
                fm_rowb, fmn_col, pm_cols, pmn_cols, lig_t, fmB = masks[s]

                # per-sample PSUM accumulators (shared tags -> slots reused
                # across samples; all matmul outputs at base partition 0)
                tAB = ppp.tile([1, K * 128], f32, tag="tAB", name=f"tAB{s}")
                lsdr = ppp.tile([1, 2, 128], f32, tag="lsdr", name=f"lsdr{s}")
                ppoolU = ppp.tile([1, 128], f32, tag="ppoolU", name=f"ppoolU{s}")

                expR_tiles = {}
                prEn_tiles = {}
                pemb_tiles = {}
                pscb_tiles = {}

                # ---------------- phase A: per R-tile ----------------
                for i in range(NT):
                    r0 = i * 128
                    LgP = kp.tile([128, K + 1, F], bf16, tag=f"lgp_{s}_{i}", name=f"lgp_{s}_{i}")
                    nc.sync.dma_start(out=LgP[:], in_=lg[s, r0:r0 + 128])
                    Pemb = LgP[:, K, :]
                    pemb_tiles[i] = Pemb

                    E = sp.tile([128, K, F], bf16, tag="E")
                    nc.scalar.activation(E[:], LgP[:, 0:K, :], Exp)

                    EW = sp.tile([128, K, F], bf16, tag="EW")
                    nc.gpsimd.tensor_mul(EW[:], E[:], wrep[:])

                    A1 = sp.tile([128, 3, F], bf16, tag="A1")
                    nc.vector.tensor_add(A1[:], E[:, 0:3, :], E[:, 3:6, :])
                    B1 = sp.tile([128, F], bf16, tag="B1")
                    nc.vector.tensor_add(B1[:], A1[:, 0, :], A1[:, 1, :])
                    nc.vector.tensor_add(B1[:], B1[:], A1[:, 2, :])
                    den = sp.tile([128, F], f32, tag="den")
                    nc.vector.tensor_add(den[:], B1[:], E[:, 6, :])

                    A2 = sp.tile([128, 3, F], bf16, tag="A2")
                    nc.vector.tensor_add(A2[:], EW[:, 0:3, :], EW[:, 3:6, :])
                    B2 = sp.tile([128, F], bf16, tag="B2")
                    nc.vector.tensor_add(B2[:], A2[:, 0, :], A2[:, 1, :])
                    nc.vector.tensor_add(B2[:], B2[:], A2[:, 2, :])
                    num = sp.tile([128, F], bf16, tag="num")
                    nc.vector.tensor_add(num[:], B2[:], EW[:, 6, :])

                    rec_f = sp.tile([128, F], f32, tag="recf")
                    nc.vector.reciprocal_approx_fast(rec_f[:], den[:])
                    rec_b = sp.tile([128, F], bf16, tag="recb")
                    nc.vector.tensor_copy(rec_b[:], rec_f[:])

                    WP = sp.tile([128, K, F], bf16, tag="WP")
                    nc.gpsimd.tensor_tensor(
                        WP[:], EW[:],
                        rec_b.unsqueeze(1).broadcast_to([128, K, F]),
                        mult,
                    )
                    nc.tensor.matmul(
                        tAB[0:1, 0:512], ones_col[:], WP[:, 0:4, :],
                        start=(i == 0), stop=(i == NT - 1),
                    )
                    nc.tensor.matmul(
                        tAB[0:1, 512:896], ones_col[:], WP[:, 4:7, :],
                        start=(i == 0), stop=(i == NT - 1),
                    )

                    # S and expR share one tile so a single PE colsum
                    # accumulates both lig_score and denR
                    SR = kp.tile([128, 2, F], bf16, tag=f"sr_{s}_{i}", name=f"sr_{s}_{i}")
                    S = SR[:, 0, :]
                    nc.vector.tensor_mul(S, num[:], rec_b[:])

                    psc = sp.tile([128, 1], f32, tag="psc")
                    nc.vector.tensor_reduce(psc[:], S, axis=mybir.AxisListType.X, op=add)
                    pscb = kp.tile([128, 1], bf16, tag=f"pscb_{s}_{i}", name=f"pscb_{s}_{i}")
                    nc.vector.tensor_copy(pscb[:], psc[:])
                    pscb_tiles[i] = pscb

                    STp = pss.tile([128, 128], bf16, tag="LCT", name=f"STp_{s}_{i}", bufs=2)
                    nc.tensor.transpose(STp[:], S, id_t[:])

                    EFT = sp.tile([128, 128], bf16, tag="EFT")
                    nc.scalar.activation(EFT[:], STp[:], Exp, bias=fmn_col[:])
                    denF = pss.tile([1, 128], f32, tag="denF")
                    nc.tensor.matmul(denF[:], ones_col[:], EFT[:], start=True, stop=True)
                    recF = sp.tile([1, 128], f32, tag="recF")
                    nc.vector.reciprocal_approx_fast(recF[:], denF[:])
                    recFTp = pss.tile([128, 1], f32, tag="denF", name=f"recFTp_{s}_{i}")
                    nc.tensor.transpose(recFTp[:], recF[:], id1f[:])
                    recFcol = sp.tile([128, 1], f32, tag="recFcol")
                    nc.vector.tensor_copy(recFcol[:], recFTp[:])

                    LCT = pss.tile([128, 128], f32, tag="LCT", bufs=2)
                    nc.tensor.matmul(LCT[:], lig_t[:], EFT[:], start=True, stop=True)
                    LCTs = sp.tile([128, 128], bf16, tag="LCTs")
                    nc.scalar.copy(LCTs[:], LCT[:])
                    PAdd = pss.tile([128, 128], f32, tag="PAdd")
                    nc.tensor.matmul(PAdd[:], LCTs[:], wr_t[:], start=True, stop=True)

                    prEn = kp.tile([128, D], bf16, tag=f"pren_{s}_{i}", name=f"pren_{s}_{i}")
                    nc.vector.tensor_scalar_mul(prEn[:], PAdd[:], recFcol[:])
                    nc.vector.tensor_add(prEn[:], prEn[:], Pemb)
                    nc.vector.tensor_add(prEn[:], prEn[:], brB[:])
                    prEn_tiles[i] = prEn

                    expR = SR[:, 1, :]
                    nc.scalar.activation(expR, S, Exp, bias=pmn_cols[:, i:i + 1])
                    expR_tiles[i] = expR
                    nc.tensor.matmul(
                        lsdr[0:1, :], ones_col[:], SR[:],
                        start=(i == 0), stop=(i == NT - 1),
                    )

                    nc.tensor.matmul(
                        ppoolU[0:1, :], pscb[:], prEn[:],
                        start=(i == 0), stop=(i == NT - 1),
                    )

                # ---------------- phase B: per sample ----------------
                recR = sm.tile([1, 128], f32, tag="recR")
                nc.vector.reciprocal_approx_fast(recR[:], lsdr[0:1, 1, :])
                recRTp = pss.tile([128, 1], f32, tag="denF", name=f"recRTp_{s}")
                nc.tensor.transpose(recRTp[:], recR[:], id1f[:])
                recRcol = sm.tile([128, 1], f32, tag="recRcol")
                nc.vector.tensor_copy(recRcol[:], recRTp[:])

                PCT = pss.tile([128, 128], f32, tag="LCT", name="PCT", bufs=2)
                for i in range(NT):
                    nc.tensor.matmul(
                        PCT[:], pemb_tiles[i], expR_tiles[i],
                        start=(i == 0), stop=(i == NT - 1),
                    )
                PCTs = sm.tile([128, 128], bf16, tag="PCTs")
                nc.scalar.copy(PCTs[:], PCT[:])
                LAdd = pss.tile([128, 128], f32, tag="PAdd", name="LAdd")
                nc.tensor.matmul(LAdd[:], PCTs[:], wf_t[:], start=True, stop=True)
                ligEn = sm.tile([F, D], bf16, tag="ligEn")
                nc.vector.tensor_scalar_mul(ligEn[:], LAdd[:], recRcol[:])
                nc.vector.tensor_add(ligEn[:], ligEn[:], lig_t[:])
                nc.vector.tensor_add(ligEn[:], ligEn[:], bfB[:])

                # lig score row -> column
                lsc = sm.tile([1, 128], bf16, tag="lsc")
                nc.vector.tensor_copy(lsc[:], lsdr[0:1, 0, :])
                lscT = pss.tile([128, 1], bf16, tag="denF", name="lscT")
                nc.tensor.transpose(lscT[:], lsc[:], id_t[0:1, 0:1])
                lsc_col = sm.tile([128, 1], bf16, tag="lsccol")
                nc.vector.tensor_copy(lsc_col[:], lscT[:])

                lpoolU = pss.tile([1, 128], f32, tag="LCT", name="lpoolU", bufs=2)
                nc.tensor.matmul(lpoolU[:], lsc_col[:], ligEn[:], start=True, stop=True)

                # totals
                psall = sm.tile([128, NT], f32, tag="psall")
                for i in range(NT):
                    nc.vector.tensor_copy(psall[:, i:i + 1], pscb_tiles[i][:])
                psum_col = sm.tile([128, 1], f32, tag="psumcol")
                nc.vector.tensor_reduce(psum_col[:], psall[:], axis=mybir.AxisListType.X, op=add)
                psum_colb = sm.tile([128, 1], bf16, tag="psumcolb")
                nc.vector.tensor_copy(psum_colb[:], psum_col[:])
                totPp = pss.tile([1, 1], f32, tag="denF", name="totPp")
                nc.tensor.matmul(totPp[:], psum_colb[:], ones_col[:], start=True, stop=True)
                totP = sm.tile([1, 1], f32, tag="totP")
                nc.vector.tensor_scalar_add(totP[:], totPp[:], 1e-8)
                totPr = sm.tile([1, 1], f32, tag="totPr")
                nc.vector.reciprocal_approx_fast(totPr[:], totP[:])

                totL = sm.tile([1, 1], f32, tag="totL")
                nc.vector.tensor_reduce(totL[:], lsc[:], axis=mybir.AxisListType.X, op=add)
                nc.vector.tensor_scalar_add(totL[:], totL[:], 1e-8)
                totLr = sm.tile([1, 1], f32, tag="totLr")
                nc.vector.reciprocal_approx_fast(totLr[:], totL[:])

                # feature row for this sample (partition 0)
                feat_s = sm.tile([1, FEAT], f32, tag="featF", name=f"featF{s}", bufs=2)
                nc.vector.tensor_mul(
                    feat_s[:, 0:D], ppoolU[0:1, :], totPr.broadcast_to([1, 128])
                )
                nc.vector.tensor_mul(
                    feat_s[:, D:2 * D], lpoolU[:], totLr.broadcast_to([1, 128])
                )
                tk = sm.tile([1, K], f32, tag="tk", name=f"tk{s}", bufs=2)
                nc.vector.tensor_reduce(
                    tk[:], tAB.rearrange("p (k f) -> p k f", k=K),
                    axis=mybir.AxisListType.X, op=add,
                )
                nc.vector.tensor_copy(feat_s[:, 2 * D:FEAT], tk[:])

                # squared norm now; the Ln/Exp(-0.5) tail is deferred past
                # the MLP so the ACT func table is switched only once
                sq = sm.tile([1, FEAT], f32, tag="sq")
                ss = sm.tile([1, 1], f32, tag="ss", name=f"ss{s}", bufs=2)
                nc.scalar.activation(sq[:], feat_s[:], Square, accum_out=ss[:])
                norm_defer.append((s, feat_s, ss))

                # transposed feature chunks for the MLP (column s)
                for c in range(3):
                    lo, hi = c * 128, min((c + 1) * 128, FEAT)
                    ftp = pss.tile([128, BC], f32, tag="PAdd", name=f"ftp{s}_{c}")
                    nc.tensor.transpose(
                        ftp[0:hi - lo, 0:1], feat_s[:, lo:hi], id1f[:]
                    )
                    nc.vector.tensor_copy(featT[c][0:hi - lo, s:s + 1], ftp[0:hi - lo, 0:1])

            # ---------------- MLP head (both samples) ----------------
            h1 = []
            for j in range(4):
                h1ps = pss.tile([128, BC], f32, tag="LCT", name=f"h1ps{j}", bufs=2)
                for c in range(3):
                    nc.tensor.matmul(
                        h1ps[:], w1_t[c][:, j * 128:(j + 1) * 128], featT[c][:],
                        start=(c == 0), stop=(c == 2),
                    )
                h1j = sm.tile([128, BC], f32, tag=f"h1_{j}", name=f"h1_{j}", bufs=1)
                nc.scalar.activation(h1j[:], h1ps[:], Relu, bias=b1_t[:, j:j + 1])
                h1.append(h1j)

            h2 = []
            for j2 in range(2):
                h2ps = pss.tile([128, BC], f32, tag="PAdd", name=f"h2ps{j2}")
                for j in range(4):
                    nc.tensor.matmul(
                        h2ps[:], w2_t[j][:, j2 * 128:(j2 + 1) * 128], h1[j][:],
                        start=(j == 0), stop=(j == 3),
                    )
                h2j = sm.tile([128, BC], f32, tag=f"h2_{j2}", name=f"h2_{j2}", bufs=1)
                last_relu = nc.scalar.activation(h2j[:], h2ps[:], Relu, bias=b2_t[:, j2:j2 + 1])
                h2.append(h2j)

            pps = pss.tile([1, BC], f32, tag="denF", name="pps")
            for j2 in range(2):
                nc.tensor.matmul(
                    pps[:], w3_t[j2][:], h2[j2][:], start=(j2 == 0), stop=(j2 == 1)
                )
            prow = sm.tile([1, BC], f32, tag="prow")
            nc.vector.tensor_scalar_add(prow[:], pps[:], float(b3))
            nc.sync.dma_start(out=predT[:], in_=prow[:])

            for s, feat_s, ss in norm_defer:
                lnss = sm.tile([1, 1], f32, tag="lnss")
                ln_i = nc.scalar.activation(lnss[:], ss[:], Ln)
                tile.add_dep_helper(
                    ln_i.ins, last_relu.ins, sync=False,
                    reason="group Ln after exp-table ACT ops")
                rsn = sm.tile([1, 1], f32, tag="rsn")
                nc.scalar.activation(rsn[:], lnss[:], Exp, scale=-0.5)
                nc.vector.tensor_scalar_min(rsn[:], rsn[:], 1e12)
                fnorm = sm.tile([1, FEAT], f32, tag="fnorm")
                nc.vector.tensor_scalar_mul(fnorm[:], feat_s[:], rsn[:])
                nc.sync.dma_start(out=featn[s:s + 1, :], in_=fnorm[:])

    nc.compile()
    return nc


def _prep_inputs(ligand_emb, protein_emb, logits, fg_mask, prot_mask,
                 type_weight, Wr, br, Wf, bf, W1, b1, W2, b2, W3, b3):
    """Host-side prep: dtype/layout transforms + per-core sharding."""
    w = _softplus64(type_weight).astype(np.float64)

    lgKF = np.empty((B, R, K + 1, F), BF16)            # [B,R,8,128]
    lgKF[:, :, 0:K, :] = np.transpose(
        np.asarray(logits, np.float32), (0, 1, 3, 2)).astype(BF16)
    lgKF[:, :, K, :] = np.asarray(protein_emb, np.float32).astype(BF16)
    lebf = np.asarray(ligand_emb, np.float32).astype(BF16)
    fm = np.asarray(fg_mask, np.float32)
    pm = np.asarray(prot_mask, np.float32)
    fmneg = ((1.0 - fm) * NEG).astype(np.float32)
    pmneg = ((1.0 - pm) * NEG).astype(np.float32)
    # pm as [B, 128, NT] columns
    pmT = np.ascontiguousarray(pm.reshape(B, NT, 128).transpose(0, 2, 1))
    pmnT = np.ascontiguousarray(pmneg.reshape(B, NT, 128).transpose(0, 2, 1))

    wrT = np.asarray(Wr, np.float32).T.astype(BF16).copy()
    wfT = np.asarray(Wf, np.float32).T.astype(BF16).copy()
    brb = np.broadcast_to(
        np.asarray(br, np.float32).reshape(1, D), (128, D)).astype(BF16).copy()
    bfb = np.broadcast_to(
        np.asarray(bf, np.float32).reshape(1, D), (128, D)).astype(BF16).copy()
    fmBrep = np.broadcast_to(
        np.asarray(fg_mask, np.float32)[:, None, :], (B, 128, F)
    ).astype(BF16).copy()

    w1T = np.zeros((384, H), np.float32)
    w1T[:FEAT] = np.asarray(W1, np.float32).T
    w1d = np.ascontiguousarray(w1T.reshape(3, 128, H))
    b1d = np.ascontiguousarray(
        np.asarray(b1, np.float32).reshape(4, 128).T
    )
    w2d = np.ascontiguousarray(np.asarray(W2, np.float32).T.reshape(4, 128, H // 2))
    b2d = np.ascontiguousarray(
        np.asarray(b2, np.float32).reshape(2, 128).T
    )
    w3d = np.ascontiguousarray(np.asarray(W3, np.float32).T.reshape(2, 128, 1))
    identity = np.eye(128, dtype=np.float32).astype(BF16)

    wrd = np.broadcast_to(
        np.asarray(w, np.float32)[None, :, None], (128, K, F)
    ).astype(BF16).copy()
    shared = dict(
        wrd=wrd, wrT=wrT, wfT=wfT, brd=brb, bfd=bfb,
        w1d=w1d, b1d=b1d, w2d=w2d, b2d=b2d, w3d=w3d, ident=identity,
    )
    in_maps = []
    for c in range(NCORES):
        sl = slice(c * BC, (c + 1) * BC)
        m = dict(shared)
        m.update(
            lg=np.ascontiguousarray(lgKF[sl]),
            le=np.ascontiguousarray(lebf[sl]),
            fmr=np.ascontiguousarray(fm[sl]),
            fmnc=np.ascontiguousarray(fmneg[sl]),
            pmT=np.ascontiguousarray(pmT[sl]),
            pmnT=np.ascontiguousarray(pmnT[sl]),
            fmBd=np.ascontiguousarray(fmBrep[sl]),
        )
        in_maps.append(m)
    return w, float(np.asarray(b3).reshape(-1)[0]), in_maps


_CACHED = {}


def kernel(**inputs):
    w, b3v, in_maps = _prep_inputs(**inputs)

    key = (tuple(np.round(w, 10)), round(b3v, 10))
    if key not in _CACHED:
        _CACHED[key] = build_program(w, b3v)
    nc = _CACHED[key]

    from concourse.bass_utils import run_bass_kernel_spmd
    res = run_bass_kernel_spmd(nc, in_maps, list(range(NCORES))).results

    pred = np.zeros((B, 1), np.float32)
    featn = np.zeros((B, FEAT), np.float32)
    for c in range(NCORES):
        pred[c * BC:(c + 1) * BC, 0] = np.asarray(res[c]["predT"], np.float32)[0]
        featn[c * BC:(c + 1) * BC] = np.asarray(res[c]["featn"], np.float32)
    return pred, featn


# revision 19
# speedup vs baseline: 1.2160x; 1.2160x over previous
"""Trainium2 Bass kernel for nn_Bidirectional_Interaction_Type_Attention.

Contract: kernel(**inputs) takes FULL unsharded inputs (numpy, as produced by
setup_inputs) and returns the FULL output tuple (pred [16,1] f32,
final_feat_norm [16,263] f32).

Sharding: data-parallel over B across 8 NeuronCores (2 samples/core). All
reductions are per-sample so no collectives are needed; small MLP params are
replicated to every core.

Per-core math (Bc=2 samples, R=1024 in 8 tiles of 128 rows):
  E  = exp(logits)              (bf16, ACT)
  EW = E * w_k                  (per-k tensor_scalar, DVE 4x)
  den/num = k-trees over E/EW   (bf16 TT adds, DVE 2x)
  S  = num * (1/den) * pm * fm
  softmax over F computed in S^T space (DMA-transpose), softmax over R in
  natural space; attention contexts + enriched embeddings via PE matmuls;
  type_strength via PE column-sum of WP = EW * (1/den); pooling via PE
  matvecs; MLP head on device.
"""

import sys
import numpy as np

for _p in ("/opt/trn_rl_repo",):
    if _p not in sys.path:
        sys.path.insert(0, _p)

import ml_dtypes

BF16 = ml_dtypes.bfloat16

B, R, F, D, K, H = 16, 1024, 128, 128, 7, 512
NCORES = 8
BC = B // NCORES          # samples per core
NT = R // 128             # R tiles per sample
FEAT = 2 * D + K          # 263
NEG = -1e9


def _softplus64(x):
    x = np.asarray(x, np.float64)
    return np.log1p(np.exp(-np.abs(x))) + np.maximum(x, 0.0)


def build_program(w, b3):
    """Build the per-core Bass program. w: softplus(type_weight) [K] float.
    b3: final bias scalar (baked as immediate)."""
    import concourse.bass as bass
    import concourse.mybir as mybir
    from concourse import bacc, tile

    f32 = mybir.dt.float32
    bf16 = mybir.dt.bfloat16
    Exp = mybir.ActivationFunctionType.Exp
    Relu = mybir.ActivationFunctionType.Relu
    Ln = mybir.ActivationFunctionType.Ln
    Square = mybir.ActivationFunctionType.Square
    mult = mybir.AluOpType.mult
    add = mybir.AluOpType.add

    nc = bacc.Bacc(None)

    # ---- DRAM I/O ----
    lg = nc.declare_dram_parameter("lg", [BC, R, K + 1, F], bf16, isOutput=False)
    le = nc.declare_dram_parameter("le", [BC, F, D], bf16, isOutput=False)
    fmr = nc.declare_dram_parameter("fmr", [BC, F], f32, isOutput=False)       # fg_mask rows
    fmnc = nc.declare_dram_parameter("fmnc", [BC, F], f32, isOutput=False)     # (1-fg)*NEG as col source
    pmT = nc.declare_dram_parameter("pmT", [BC, 128, NT], f32, isOutput=False)   # pm[s] cols per tile
    pmnT = nc.declare_dram_parameter("pmnT", [BC, 128, NT], f32, isOutput=False)  # (1-pm)*NEG cols
    wrT = nc.declare_dram_parameter("wrT", [D, D], bf16, isOutput=False)   # Wr^T
    wfT = nc.declare_dram_parameter("wfT", [D, D], bf16, isOutput=False)   # Wf^T
    brd = nc.declare_dram_parameter("brd", [128, D], bf16, isOutput=False)
    bfd = nc.declare_dram_parameter("bfd", [128, D], bf16, isOutput=False)
    fmBd = nc.declare_dram_parameter("fmBd", [BC, 128, F], bf16, isOutput=False)
    wrd = nc.declare_dram_parameter("wrd", [128, K, F], bf16, isOutput=False)
    w1d = nc.declare_dram_parameter("w1d", [3, 128, H], f32, isOutput=False)   # W1^T padded 263->384
    b1d = nc.declare_dram_parameter("b1d", [128, 4], f32, isOutput=False)       # b1 col-chunks
    w2d = nc.declare_dram_parameter("w2d", [4, 128, H // 2], f32, isOutput=False)  # W2^T chunks
    b2d = nc.declare_dram_parameter("b2d", [128, 2], f32, isOutput=False)
    w3d = nc.declare_dram_parameter("w3d", [2, 128, 1], f32, isOutput=False)   # W3^T chunks
    ident = nc.declare_dram_parameter("ident", [128, 128], bf16, isOutput=False)

    predT = nc.declare_dram_parameter("predT", [1, BC], f32, isOutput=True)
    featn = nc.declare_dram_parameter("featn", [BC, FEAT], f32, isOutput=True)

    with tile.TileContext(nc) as tc:
        with (
            tc.tile_pool(name="const", bufs=1) as cpool,
            tc.tile_pool(name="stream", bufs=3) as sp,
            tc.tile_pool(name="small", bufs=3) as sm,
            tc.tile_pool(name="keep", bufs=1, space="SBUF") as kp,
            tc.tile_pool(name="samp", bufs=1) as smp,
            tc.tile_pool(name="ps_persist", bufs=1, space="PSUM") as ppp,
            tc.tile_pool(name="ps_stream", bufs=1, space="PSUM") as pss,
        ):
            # ---- constants ----
            ones_col = cpool.tile([128, 1], bf16, tag="ones")
            nc.vector.memset(ones_col[:], 1.0)
            id_t = cpool.tile([128, 128], bf16, tag="ident")
            nc.sync.dma_start(out=id_t[:], in_=ident[:])
            wr_t = cpool.tile([D, D], bf16, tag="wrT")
            nc.sync.dma_start(out=wr_t[:], in_=wrT[:])
            wf_t = cpool.tile([D, D], bf16, tag="wfT")
            nc.sync.dma_start(out=wf_t[:], in_=wfT[:])
            brB = cpool.tile([128, D], bf16, tag="brB")
            nc.sync.dma_start(out=brB[:], in_=brd[:])
            bfB = cpool.tile([128, D], bf16, tag="bfB")
            nc.sync.dma_start(out=bfB[:], in_=bfd[:])
            id1f = cpool.tile([1, 1], f32, tag="id1f")
            nc.vector.memset(id1f[:], 1.0)
            wrep = cpool.tile([128, K, F], bf16, tag="wrep")
            nc.sync.dma_start(out=wrep[:], in_=wrd[:])
            w1_t = [cpool.tile([128, H], f32, tag=f"w1_{c}", name=f"w1_{c}") for c in range(3)]
            for c in range(3):
                nc.sync.dma_start(out=w1_t[c][:], in_=w1d[c])
            b1_t = cpool.tile([128, 4], f32, tag="b1")
            nc.sync.dma_start(out=b1_t[:], in_=b1d[:])
            w2_t = [cpool.tile([128, H // 2], f32, tag=f"w2_{j}", name=f"w2_{j}") for j in range(4)]
            for j in range(4):
                nc.sync.dma_start(out=w2_t[j][:], in_=w2d[j])
            b2_t = cpool.tile([128, 2], f32, tag="b2")
            nc.sync.dma_start(out=b2_t[:], in_=b2d[:])
            w3_t = [cpool.tile([128, 1], f32, tag=f"w3_{j}", name=f"w3_{j}") for j in range(2)]
            for j in range(2):
                nc.sync.dma_start(out=w3_t[j][:], in_=w3d[j])

            # ---- per-sample masks/ligand ----
            masks = {}
            for s in range(BC):
                fm_row = smp.tile([1, F], f32, tag=f"fmrow{s}", name=f"fmrow{s}")
                nc.sync.dma_start(out=fm_row[:], in_=fmr[s].unsqueeze(0))
                fm_rowb = smp.tile([1, F], bf16, tag=f"fmrowb{s}", name=f"fmrowb{s}")
                nc.vector.tensor_copy(fm_rowb[:], fm_row[:])
                fmn_col = smp.tile([128, 1], f32, tag=f"fmncol{s}", name=f"fmncol{s}")
                nc.sync.dma_start(out=fmn_col[:], in_=fmnc[s].unsqueeze(1))
                pm_cols = smp.tile([128, NT], f32, tag=f"pmcols{s}", name=f"pmcols{s}")
                nc.sync.dma_start(out=pm_cols[:], in_=pmT[s])
                pmn_cols = smp.tile([128, NT], f32, tag=f"pmncols{s}", name=f"pmncols{s}")
                nc.sync.dma_start(out=pmn_cols[:], in_=pmnT[s])
                lig_t = smp.tile([F, D], bf16, tag=f"lig{s}", name=f"lig{s}")
                nc.sync.dma_start(out=lig_t[:], in_=le[s])
                fmB = smp.tile([128, F], bf16, tag=f"fmB{s}", name=f"fmB{s}")
                nc.sync.dma_start(out=fmB[:], in_=fmBd[s])
                masks[s] = (fm_rowb, fmn_col, pm_cols, pmn_cols, lig_t, fmB)

            featT = [sm.tile([128, BC], f32, tag=f"ft{c}", name=f"ft{c}", bufs=1)
                     for c in range(3)]
            norm_defer = []
            nc.vector.memset(featT[2][:], 0.0)

            for s in range(BC):
                fm_rowb, fmn_col, pm_cols, pmn_cols, lig_t, fmB = masks[s]

                # per-sample PSUM accumulators (shared tags -> slots reused
                # across samples; all matmul outputs at base partition 0)
                tAB = ppp.tile([1, K * 128], f32, tag="tAB", name=f"tAB{s}")
                lsdr = ppp.tile([1, 2, 128], f32, tag="lsdr", name=f"lsdr{s}")
                ppoolU = ppp.tile([1, 128], f32, tag="ppoolU", name=f"ppoolU{s}")

                expR_tiles = {}
                prEn_tiles = {}
                pemb_tiles = {}
                pscb_tiles = {}

                # ---------------- phase A: per R-tile ----------------
                for i in range(NT):
                    r0 = i * 128
                    LgP = kp.tile([128, K + 1, F], bf16, tag=f"lgp_{s}_{i}", name=f"lgp_{s}_{i}")
                    nc.sync.dma_start(out=LgP[:], in_=lg[s, r0:r0 + 128])
                    Pemb = LgP[:, K, :]
                    pemb_tiles[i] = Pemb

                    # X holds E (slot 0) and EW (slot 1); den/num trees run
                    # on both halves at once (4 wide ops instead of 8)
                    X = sp.tile([128, 2, K, F], bf16, tag="X")
                    E = X[:, 0]
                    EW = X[:, 1]
                    nc.scalar.activation(E, LgP[:, 0:K, :], Exp)
                    nc.vector.tensor_mul(EW, E, wrep[:])

                    P4 = sp.tile([128, 2, 4, F], bf16, tag="P4")
                    nc.vector.tensor_add(
                        P4[:, :, 0:3, :], X[:, :, 0:3, :], X[:, :, 4:7, :])
                    nc.vector.tensor_copy(P4[:, :, 3, :], X[:, :, 3, :])
                    Q2 = sp.tile([128, 2, 2, F], bf16, tag="Q2")
                    nc.vector.tensor_add(
                        Q2[:], P4[:, :, 0:2, :], P4[:, :, 2:4, :])
                    dennum = sp.tile([128, 2, F], f32, tag="dennum")
                    nc.vector.tensor_add(
                        dennum[:], Q2[:, :, 0, :], Q2[:, :, 1, :])
                    den = dennum[:, 0, :]
                    num = dennum[:, 1, :]

                    rec_f = sp.tile([128, F], f32, tag="recf")
                    nc.vector.reciprocal_approx_fast(rec_f[:], den)
                    rec_b = sp.tile([128, F], bf16, tag="recb")
                    nc.vector.tensor_copy(rec_b[:], rec_f[:])

                    WP = sp.tile([128, K, F], bf16, tag="WP")
                    nc.vector.tensor_tensor(
                        WP[:], EW,
                        rec_b.unsqueeze(1).broadcast_to([128, K, F]),
                        mult,
                    )
                    nc.tensor.matmul(
                        tAB[0:1, 0:512], ones_col[:], WP[:, 0:4, :],
                        start=(i == 0), stop=(i == NT - 1),
                    )
                    nc.tensor.matmul(
                        tAB[0:1, 512:896], ones_col[:], WP[:, 4:7, :],
                        start=(i == 0), stop=(i == NT - 1),
                    )

                    # S and expR share one tile so a single PE colsum
                    # accumulates both lig_score and denR
                    SR = kp.tile([128, 2, F], bf16, tag=f"sr_{s}_{i}", name=f"sr_{s}_{i}")
                    S = SR[:, 0, :]
                    nc.vector.tensor_mul(S, num, rec_b[:])

                    psc = sp.tile([128, 1], f32, tag="psc")
                    nc.vector.tensor_reduce(psc[:], S, axis=mybir.AxisListType.X, op=add)
                    pscb = kp.tile([128, 1], bf16, tag=f"pscb_{s}_{i}", name=f"pscb_{s}_{i}")
                    nc.vector.tensor_copy(pscb[:], psc[:])
                    pscb_tiles[i] = pscb

                    STp = pss.tile([128, 128], bf16, tag="LCT", name=f"STp_{s}_{i}", bufs=2)
                    nc.tensor.transpose(STp[:], S, id_t[:])

                    EFT = sp.tile([128, 128], bf16, tag="EFT")
                    nc.scalar.activation(EFT[:], STp[:], Exp, bias=fmn_col[:])
                    denF = pss.tile([1, 128], f32, tag="LCT", name=f"denF_{s}_{i}", bufs=2)
                    nc.tensor.matmul(denF[:], ones_col[:], EFT[:], start=True, stop=True)
                    recF = sp.tile([1, 128], f32, tag="recF")
                    nc.vector.reciprocal_approx_fast(recF[:], denF[:])
                    recFb = sp.tile([1, 128], bf16, tag="recFb")
                    nc.vector.tensor_copy(recFb[:], recF[:])
                    recFB = sp.tile([128, 128], bf16, tag="recFB")
                    nc.gpsimd.partition_broadcast(recFB[:], recFb[:])
                    attnFT = sp.tile([128, 128], bf16, tag="attnFT")
                    nc.vector.tensor_mul(attnFT[:], EFT[:], recFB[:])

                    LCT = pss.tile([128, 128], f32, tag="LCT", bufs=2)
                    nc.tensor.matmul(LCT[:], lig_t[:], attnFT[:], start=True, stop=True)
                    LCTs = sp.tile([128, 128], bf16, tag="LCTs")
                    nc.scalar.copy(LCTs[:], LCT[:])
                    PAdd = pss.tile([128, 128], f32, tag="PAdd", bufs=2)
                    nc.tensor.matmul(PAdd[:], LCTs[:], wr_t[:], start=True, stop=True)

                    prEn = kp.tile([128, D], bf16, tag=f"pren_{s}_{i}", name=f"pren_{s}_{i}")
                    nc.vector.tensor_add(prEn[:], PAdd[:], Pemb)
                    nc.vector.tensor_add(prEn[:], prEn[:], brB[:])
                    prEn_tiles[i] = prEn

                    expR = SR[:, 1, :]
                    nc.scalar.activation(expR, S, Exp, bias=pmn_cols[:, i:i + 1])
                    expR_tiles[i] = expR
                    nc.tensor.matmul(
                        lsdr[0:1, :], ones_col[:], SR[:],
                        start=(i == 0), stop=(i == NT - 1),
                    )

                    nc.tensor.matmul(
                        ppoolU[0:1, :], pscb[:], prEn[:],
                        start=(i == 0), stop=(i == NT - 1),
                    )

                # ---------------- phase B: per sample ----------------
                recR = sm.tile([1, 128], f32, tag="recR")
                nc.vector.reciprocal_approx_fast(recR[:], lsdr[0:1, 1, :])
                recRTp = pss.tile([128, 1], f32, tag="PAdd", name=f"recRTp_{s}", bufs=2)
                nc.tensor.transpose(recRTp[:], recR[:], id1f[:])
                recRcol = sm.tile([128, 1], f32, tag="recRcol")
                nc.vector.tensor_copy(recRcol[:], recRTp[:])

                PCT = pss.tile([128, 128], f32, tag="LCT", name="PCT", bufs=2)
                for i in range(NT):
                    nc.tensor.matmul(
                        PCT[:], pemb_tiles[i], expR_tiles[i],
                        start=(i == 0), stop=(i == NT - 1),
                    )
                PCTs = sm.tile([128, 128], bf16, tag="PCTs")
                nc.scalar.copy(PCTs[:], PCT[:])
                LAdd = pss.tile([128, 128], f32, tag="PAdd", name="LAdd", bufs=2)
                nc.tensor.matmul(LAdd[:], PCTs[:], wf_t[:], start=True, stop=True)
                ligEn = sm.tile([F, D], bf16, tag="ligEn")
                nc.vector.tensor_scalar_mul(ligEn[:], LAdd[:], recRcol[:])
                nc.vector.tensor_add(ligEn[:], ligEn[:], lig_t[:])
                nc.vector.tensor_add(ligEn[:], ligEn[:], bfB[:])

                # lig score row -> column
                lsc = sm.tile([1, 128], bf16, tag="lsc")
                nc.vector.tensor_copy(lsc[:], lsdr[0:1, 0, :])
                lscT = pss.tile([128, 1], bf16, tag="PAdd", name="lscT", bufs=2)
                nc.tensor.transpose(lscT[:], lsc[:], id_t[0:1, 0:1])
                lsc_col = sm.tile([128, 1], bf16, tag="lsccol")
                nc.vector.tensor_copy(lsc_col[:], lscT[:])

                lpoolU = pss.tile([1, 128], f32, tag="LCT", name="lpoolU", bufs=2)
                nc.tensor.matmul(lpoolU[:], lsc_col[:], ligEn[:], start=True, stop=True)

                # totals
                psall = sm.tile([128, NT], f32, tag="psall")
                for i in range(NT):
                    nc.vector.tensor_copy(psall[:, i:i + 1], pscb_tiles[i][:])
                psum_col = sm.tile([128, 1], f32, tag="psumcol")
                nc.vector.tensor_reduce(psum_col[:], psall[:], axis=mybir.AxisListType.X, op=add)
                psum_colb = sm.tile([128, 1], bf16, tag="psumcolb")
                nc.vector.tensor_copy(psum_colb[:], psum_col[:])
                totPp = pss.tile([1, 1], f32, tag="PAdd", name="totPp", bufs=2)
                nc.tensor.matmul(totPp[:], psum_colb[:], ones_col[:], start=True, stop=True)
                totP = sm.tile([1, 1], f32, tag="totP")
                nc.vector.tensor_scalar_add(totP[:], totPp[:], 1e-8)
                totPr = sm.tile([1, 1], f32, tag="totPr")
                nc.vector.reciprocal_approx_fast(totPr[:], totP[:])

                totL = sm.tile([1, 1], f32, tag="totL")
                nc.vector.tensor_reduce(totL[:], lsc[:], axis=mybir.AxisListType.X, op=add)
                nc.vector.tensor_scalar_add(totL[:], totL[:], 1e-8)
                totLr = sm.tile([1, 1], f32, tag="totLr")
                nc.vector.reciprocal_approx_fast(totLr[:], totL[:])

                # feature row for this sample (partition 0)
                feat_s = sm.tile([1, FEAT], f32, tag="featF", name=f"featF{s}", bufs=2)
                nc.vector.tensor_mul(
                    feat_s[:, 0:D], ppoolU[0:1, :], totPr.broadcast_to([1, 128])
                )
                nc.vector.tensor_mul(
                    feat_s[:, D:2 * D], lpoolU[:], totLr.broadcast_to([1, 128])
                )
                tk = sm.tile([1, K], f32, tag="tk", name=f"tk{s}", bufs=2)
                nc.vector.tensor_reduce(
                    tk[:], tAB.rearrange("p (k f) -> p k f", k=K),
                    axis=mybir.AxisListType.X, op=add,
                )
                nc.vector.tensor_copy(feat_s[:, 2 * D:FEAT], tk[:])

                # squared norm now; the Ln/Exp(-0.5) tail is deferred past
                # the MLP so the ACT func table is switched only once
                sq = sm.tile([1, FEAT], f32, tag="sq")
                ss = sm.tile([1, 1], f32, tag="ss", name=f"ss{s}", bufs=2)
                nc.scalar.activation(sq[:], feat_s[:], Square, accum_out=ss[:])
                norm_defer.append((s, feat_s, ss))

                # transposed feature chunks for the MLP (column s)
                for c in range(3):
                    lo, hi = c * 128, min((c + 1) * 128, FEAT)
                    ftp = pss.tile([128, BC], f32, tag="PAdd", name=f"ftp{s}_{c}", bufs=2)
                    nc.tensor.transpose(
                        ftp[0:hi - lo, 0:1], feat_s[:, lo:hi], id1f[:]
                    )
                    nc.vector.tensor_copy(featT[c][0:hi - lo, s:s + 1], ftp[0:hi - lo, 0:1])

            # ---------------- MLP head (both samples) ----------------
            h1 = []
            for j in range(4):
                h1ps = pss.tile([128, BC], f32, tag="LCT", name=f"h1ps{j}", bufs=2)
                for c in range(3):
                    nc.tensor.matmul(
                        h1ps[:], w1_t[c][:, j * 128:(j + 1) * 128], featT[c][:],
                        start=(c == 0), stop=(c == 2),
                    )
                h1j = sm.tile([128, BC], f32, tag=f"h1_{j}", name=f"h1_{j}", bufs=1)
                nc.scalar.activation(h1j[:], h1ps[:], Relu, bias=b1_t[:, j:j + 1])
                h1.append(h1j)

            h2 = []
            for j2 in range(2):
                h2ps = pss.tile([128, BC], f32, tag="PAdd", name=f"h2ps{j2}", bufs=2)
                for j in range(4):
                    nc.tensor.matmul(
                        h2ps[:], w2_t[j][:, j2 * 128:(j2 + 1) * 128], h1[j][:],
                        start=(j == 0), stop=(j == 3),
                    )
                h2j = sm.tile([128, BC], f32, tag=f"h2_{j2}", name=f"h2_{j2}", bufs=1)
                last_relu = nc.scalar.activation(h2j[:], h2ps[:], Relu, bias=b2_t[:, j2:j2 + 1])
                h2.append(h2j)

            pps = pss.tile([1, BC], f32, tag="PAdd", name="pps", bufs=2)
            for j2 in range(2):
                nc.tensor.matmul(
                    pps[:], w3_t[j2][:], h2[j2][:], start=(j2 == 0), stop=(j2 == 1)
                )
            prow = sm.tile([1, BC], f32, tag="prow")
            nc.vector.tensor_scalar_add(prow[:], pps[:], float(b3))
            nc.sync.dma_start(out=predT[:], in_=prow[:])

            for s, feat_s, ss in norm_defer:
                lnss = sm.tile([1, 1], f32, tag="lnss")
                ln_i = nc.scalar.activation(lnss[:], ss[:], Ln)
                tile.add_dep_helper(
                    ln_i.ins, last_relu.ins, sync=False,
                    reason="group Ln after exp-table ACT ops")
                rsn = sm.tile([1, 1], f32, tag="rsn")
                nc.scalar.activation(rsn[:], lnss[:], Exp, scale=-0.5)
                nc.vector.tensor_scalar_min(rsn[:], rsn[:], 1e12)
                fnorm = sm.tile([1, FEAT], f32, tag="fnorm")
                nc.vector.tensor_scalar_mul(fnorm[:], feat_s[:], rsn[:])
                nc.sync.dma_start(out=featn[s:s + 1, :], in_=fnorm[:])

    nc.compile()
    return nc


def _prep_inputs(ligand_emb, protein_emb, logits, fg_mask, prot_mask,
                 type_weight, Wr, br, Wf, bf, W1, b1, W2, b2, W3, b3):
    """Host-side prep: dtype/layout transforms + per-core sharding."""
    w = _softplus64(type_weight).astype(np.float64)

    lgKF = np.empty((B, R, K + 1, F), BF16)            # [B,R,8,128]
    lgKF[:, :, 0:K, :] = np.transpose(
        np.asarray(logits, np.float32), (0, 1, 3, 2)).astype(BF16)
    lgKF[:, :, K, :] = np.asarray(protein_emb, np.float32).astype(BF16)
    lebf = np.asarray(ligand_emb, np.float32).astype(BF16)
    fm = np.asarray(fg_mask, np.float32)
    pm = np.asarray(prot_mask, np.float32)
    fmneg = ((1.0 - fm) * NEG).astype(np.float32)
    pmneg = ((1.0 - pm) * NEG).astype(np.float32)
    # pm as [B, 128, NT] columns
    pmT = np.ascontiguousarray(pm.reshape(B, NT, 128).transpose(0, 2, 1))
    pmnT = np.ascontiguousarray(pmneg.reshape(B, NT, 128).transpose(0, 2, 1))

    wrT = np.asarray(Wr, np.float32).T.astype(BF16).copy()
    wfT = np.asarray(Wf, np.float32).T.astype(BF16).copy()
    brb = np.broadcast_to(
        np.asarray(br, np.float32).reshape(1, D), (128, D)).astype(BF16).copy()
    bfb = np.broadcast_to(
        np.asarray(bf, np.float32).reshape(1, D), (128, D)).astype(BF16).copy()
    fmBrep = np.broadcast_to(
        np.asarray(fg_mask, np.float32)[:, None, :], (B, 128, F)
    ).astype(BF16).copy()

    w1T = np.zeros((384, H), np.float32)
    w1T[:FEAT] = np.asarray(W1, np.float32).T
    w1d = np.ascontiguousarray(w1T.reshape(3, 128, H))
    b1d = np.ascontiguousarray(
        np.asarray(b1, np.float32).reshape(4, 128).T
    )
    w2d = np.ascontiguousarray(np.asarray(W2, np.float32).T.reshape(4, 128, H // 2))
    b2d = np.ascontiguousarray(
        np.asarray(b2, np.float32).reshape(2, 128).T
    )
    w3d = np.ascontiguousarray(np.asarray(W3, np.float32).T.reshape(2, 128, 1))
    identity = np.eye(128, dtype=np.float32).astype(BF16)

    wrd = np.broadcast_to(
        np.asarray(w, np.float32)[None, :, None], (128, K, F)
    ).astype(BF16).copy()
    shared = dict(
        wrd=wrd, wrT=wrT, wfT=wfT, brd=brb, bfd=bfb,
        w1d=w1d, b1d=b1d, w2d=w2d, b2d=b2d, w3d=w3d, ident=identity,
    )
    in_maps = []
    for c in range(NCORES):
        sl = slice(c * BC, (c + 1) * BC)
        m = dict(shared)
        m.update(
            lg=np.ascontiguousarray(lgKF[sl]),
            le=np.ascontiguousarray(lebf[sl]),
            fmr=np.ascontiguousarray(fm[sl]),
            fmnc=np.ascontiguousarray(fmneg[sl]),
            pmT=np.ascontiguousarray(pmT[sl]),
            pmnT=np.ascontiguousarray(pmnT[sl]),
            fmBd=np.ascontiguousarray(fmBrep[sl]),
        )
        in_maps.append(m)
    return w, float(np.asarray(b3).reshape(-1)[0]), in_maps


_CACHED = {}


def kernel(**inputs):
    w, b3v, in_maps = _prep_inputs(**inputs)

    key = (tuple(np.round(w, 10)), round(b3v, 10))
    if key not in _CACHED:
        _CACHED[key] = build_program(w, b3v)
    nc = _CACHED[key]

    from concourse.bass_utils import run_bass_kernel_spmd
    res = run_bass_kernel_spmd(nc, in_maps, list(range(NCORES))).results

    pred = np.zeros((B, 1), np.float32)
    featn = np.zeros((B, FEAT), np.float32)
    for c in range(NCORES):
        pred[c * BC:(c + 1) * BC, 0] = np.asarray(res[c]["predT"], np.float32)[0]
        featn[c * BC:(c + 1) * BC] = np.asarray(res[c]["featn"], np.float32)
    return pred, featn


# revision 20
# speedup vs baseline: 1.2375x; 1.0177x over previous
"""Trainium2 Bass kernel for nn_Bidirectional_Interaction_Type_Attention.

Contract: kernel(**inputs) takes FULL unsharded inputs (numpy, as produced by
setup_inputs) and returns the FULL output tuple (pred [16,1] f32,
final_feat_norm [16,263] f32).

Sharding: data-parallel over B across 8 NeuronCores (2 samples/core). All
reductions are per-sample so no collectives are needed; small MLP params are
replicated to every core.

Per-core math (Bc=2 samples, R=1024 in 8 tiles of 128 rows):
  E  = exp(logits)              (bf16, ACT)
  EW = E * w_k                  (per-k tensor_scalar, DVE 4x)
  den/num = k-trees over E/EW   (bf16 TT adds, DVE 2x)
  S  = num * (1/den) * pm * fm
  softmax over F computed in S^T space (DMA-transpose), softmax over R in
  natural space; attention contexts + enriched embeddings via PE matmuls;
  type_strength via PE column-sum of WP = EW * (1/den); pooling via PE
  matvecs; MLP head on device.
"""

import sys
import numpy as np

for _p in ("/opt/trn_rl_repo",):
    if _p not in sys.path:
        sys.path.insert(0, _p)

import ml_dtypes

BF16 = ml_dtypes.bfloat16

B, R, F, D, K, H = 16, 1024, 128, 128, 7, 512
NCORES = 8
BC = B // NCORES          # samples per core
NT = R // 128             # R tiles per sample
FEAT = 2 * D + K          # 263
NEG = -1e9


def _softplus64(x):
    x = np.asarray(x, np.float64)
    return np.log1p(np.exp(-np.abs(x))) + np.maximum(x, 0.0)


def build_program(w, b3):
    """Build the per-core Bass program. w: softplus(type_weight) [K] float.
    b3: final bias scalar (baked as immediate)."""
    import concourse.bass as bass
    import concourse.mybir as mybir
    from concourse import bacc, tile

    f32 = mybir.dt.float32
    bf16 = mybir.dt.bfloat16
    Exp = mybir.ActivationFunctionType.Exp
    Relu = mybir.ActivationFunctionType.Relu
    Ln = mybir.ActivationFunctionType.Ln
    Square = mybir.ActivationFunctionType.Square
    mult = mybir.AluOpType.mult
    add = mybir.AluOpType.add

    nc = bacc.Bacc(None)

    # ---- DRAM I/O ----
    lg = nc.declare_dram_parameter("lg", [BC, R, K + 1, F], bf16, isOutput=False)
    le = nc.declare_dram_parameter("le", [BC, F, D], bf16, isOutput=False)
    fmr = nc.declare_dram_parameter("fmr", [BC, F], f32, isOutput=False)       # fg_mask rows
    fmnc = nc.declare_dram_parameter("fmnc", [BC, F], f32, isOutput=False)     # (1-fg)*NEG as col source
    pmT = nc.declare_dram_parameter("pmT", [BC, 128, NT], f32, isOutput=False)   # pm[s] cols per tile
    pmnT = nc.declare_dram_parameter("pmnT", [BC, 128, NT], f32, isOutput=False)  # (1-pm)*NEG cols
    wrT = nc.declare_dram_parameter("wrT", [D, D], bf16, isOutput=False)   # Wr^T
    wfT = nc.declare_dram_parameter("wfT", [D, D], bf16, isOutput=False)   # Wf^T
    brd = nc.declare_dram_parameter("brd", [128, D], bf16, isOutput=False)
    bfd = nc.declare_dram_parameter("bfd", [128, D], bf16, isOutput=False)
    fmBd = nc.declare_dram_parameter("fmBd", [BC, 128, F], bf16, isOutput=False)
    wrd = nc.declare_dram_parameter("wrd", [128, K, F], bf16, isOutput=False)
    w1d = nc.declare_dram_parameter("w1d", [3, 128, H], f32, isOutput=False)   # W1^T padded 263->384
    b1d = nc.declare_dram_parameter("b1d", [128, 4], f32, isOutput=False)       # b1 col-chunks
    w2d = nc.declare_dram_parameter("w2d", [4, 128, H // 2], f32, isOutput=False)  # W2^T chunks
    b2d = nc.declare_dram_parameter("b2d", [128, 2], f32, isOutput=False)
    w3d = nc.declare_dram_parameter("w3d", [2, 128, 1], f32, isOutput=False)   # W3^T chunks
    ident = nc.declare_dram_parameter("ident", [128, 128], bf16, isOutput=False)

    predT = nc.declare_dram_parameter("predT", [1, BC], f32, isOutput=True)
    featn = nc.declare_dram_parameter("featn", [BC, FEAT], f32, isOutput=True)

    with tile.TileContext(nc) as tc:
        with (
            tc.tile_pool(name="const", bufs=1) as cpool,
            tc.tile_pool(name="stream", bufs=3) as sp,
            tc.tile_pool(name="small", bufs=3) as sm,
            tc.tile_pool(name="keep", bufs=1, space="SBUF") as kp,
            tc.tile_pool(name="samp", bufs=1) as smp,
            tc.tile_pool(name="ps_persist", bufs=1, space="PSUM") as ppp,
            tc.tile_pool(name="ps_stream", bufs=1, space="PSUM") as pss,
        ):
            # ---- constants ----
            ones_col = cpool.tile([128, 1], bf16, tag="ones")
            nc.vector.memset(ones_col[:], 1.0)
            id_t = cpool.tile([128, 128], bf16, tag="ident")
            nc.gpsimd.dma_start(out=id_t[:], in_=ident[:])
            wr_t = cpool.tile([D, D], bf16, tag="wrT")
            nc.gpsimd.dma_start(out=wr_t[:], in_=wrT[:])
            wf_t = cpool.tile([D, D], bf16, tag="wfT")
            nc.gpsimd.dma_start(out=wf_t[:], in_=wfT[:])
            brB = cpool.tile([128, D], bf16, tag="brB")
            nc.gpsimd.dma_start(out=brB[:], in_=brd[:])
            bfB = cpool.tile([128, D], bf16, tag="bfB")
            nc.gpsimd.dma_start(out=bfB[:], in_=bfd[:])
            id1f = cpool.tile([1, 1], f32, tag="id1f")
            nc.vector.memset(id1f[:], 1.0)
            wrep = cpool.tile([128, K, F], bf16, tag="wrep")
            nc.gpsimd.dma_start(out=wrep[:], in_=wrd[:])
            w1_t = [cpool.tile([128, H], f32, tag=f"w1_{c}", name=f"w1_{c}") for c in range(3)]
            for c in range(3):
                nc.gpsimd.dma_start(out=w1_t[c][:], in_=w1d[c])
            b1_t = cpool.tile([128, 4], f32, tag="b1")
            nc.gpsimd.dma_start(out=b1_t[:], in_=b1d[:])
            w2_t = [cpool.tile([128, H // 2], f32, tag=f"w2_{j}", name=f"w2_{j}") for j in range(4)]
            for j in range(4):
                nc.gpsimd.dma_start(out=w2_t[j][:], in_=w2d[j])
            b2_t = cpool.tile([128, 2], f32, tag="b2")
            nc.gpsimd.dma_start(out=b2_t[:], in_=b2d[:])
            w3_t = [cpool.tile([128, 1], f32, tag=f"w3_{j}", name=f"w3_{j}") for j in range(2)]
            for j in range(2):
                nc.gpsimd.dma_start(out=w3_t[j][:], in_=w3d[j])

            # ---- per-sample masks/ligand ----
            masks = {}
            for s in range(BC):
                fm_row = smp.tile([1, F], f32, tag=f"fmrow{s}", name=f"fmrow{s}")
                nc.gpsimd.dma_start(out=fm_row[:], in_=fmr[s].unsqueeze(0))
                fm_rowb = smp.tile([1, F], bf16, tag=f"fmrowb{s}", name=f"fmrowb{s}")
                nc.vector.tensor_copy(fm_rowb[:], fm_row[:])
                fmn_col = smp.tile([128, 1], f32, tag=f"fmncol{s}", name=f"fmncol{s}")
                nc.gpsimd.dma_start(out=fmn_col[:], in_=fmnc[s].unsqueeze(1))
                pm_cols = smp.tile([128, NT], f32, tag=f"pmcols{s}", name=f"pmcols{s}")
                nc.gpsimd.dma_start(out=pm_cols[:], in_=pmT[s])
                pmn_cols = smp.tile([128, NT], f32, tag=f"pmncols{s}", name=f"pmncols{s}")
                nc.gpsimd.dma_start(out=pmn_cols[:], in_=pmnT[s])
                lig_t = smp.tile([F, D], bf16, tag=f"lig{s}", name=f"lig{s}")
                nc.gpsimd.dma_start(out=lig_t[:], in_=le[s])
                fmB = smp.tile([128, F], bf16, tag=f"fmB{s}", name=f"fmB{s}")
                nc.gpsimd.dma_start(out=fmB[:], in_=fmBd[s])
                masks[s] = (fm_rowb, fmn_col, pm_cols, pmn_cols, lig_t, fmB)

            featT = [sm.tile([128, BC], f32, tag=f"ft{c}", name=f"ft{c}", bufs=1)
                     for c in range(3)]
            norm_defer = []
            nc.vector.memset(featT[2][:], 0.0)

            for s in range(BC):
                fm_rowb, fmn_col, pm_cols, pmn_cols, lig_t, fmB = masks[s]

                # per-sample PSUM accumulators (shared tags -> slots reused
                # across samples; all matmul outputs at base partition 0)
                tAB = ppp.tile([1, K * 128], f32, tag="tAB", name=f"tAB{s}")
                lsdr = ppp.tile([1, 2, 128], f32, tag="lsdr", name=f"lsdr{s}")
                ppoolU = ppp.tile([1, 128], f32, tag="ppoolU", name=f"ppoolU{s}")

                expR_tiles = {}
                prEn_tiles = {}
                pemb_tiles = {}
                pscb_tiles = {}

                # ---------------- phase A: per R-tile ----------------
                for i in range(NT):
                    r0 = i * 128
                    LgP = kp.tile([128, K + 1, F], bf16, tag=f"lgp_{s}_{i}", name=f"lgp_{s}_{i}")
                    nc.sync.dma_start(out=LgP[:], in_=lg[s, r0:r0 + 128])
                    Pemb = LgP[:, K, :]
                    pemb_tiles[i] = Pemb

                    # X holds E (slot 0) and EW (slot 1); den/num trees run
                    # on both halves at once (4 wide ops instead of 8)
                    X = sp.tile([128, 2, K, F], bf16, tag="X")
                    E = X[:, 0]
                    EW = X[:, 1]
                    nc.scalar.activation(E, LgP[:, 0:K, :], Exp)
                    nc.vector.tensor_mul(EW, E, wrep[:])

                    P4 = sp.tile([128, 2, 4, F], bf16, tag="P4")
                    nc.vector.tensor_add(
                        P4[:, :, 0:3, :], X[:, :, 0:3, :], X[:, :, 4:7, :])
                    nc.vector.tensor_copy(P4[:, :, 3, :], X[:, :, 3, :])
                    Q2 = sp.tile([128, 2, 2, F], bf16, tag="Q2")
                    nc.vector.tensor_add(
                        Q2[:], P4[:, :, 0:2, :], P4[:, :, 2:4, :])
                    dennum = sp.tile([128, 2, F], f32, tag="dennum")
                    nc.vector.tensor_add(
                        dennum[:], Q2[:, :, 0, :], Q2[:, :, 1, :])
                    den = dennum[:, 0, :]
                    num = dennum[:, 1, :]

                    rec_f = sp.tile([128, F], f32, tag="recf")
                    nc.vector.reciprocal_approx_fast(rec_f[:], den)
                    rec_b = sp.tile([128, F], bf16, tag="recb")
                    nc.vector.tensor_copy(rec_b[:], rec_f[:])

                    WP = sp.tile([128, K, F], bf16, tag="WP")
                    nc.vector.tensor_tensor(
                        WP[:], EW,
                        rec_b.unsqueeze(1).broadcast_to([128, K, F]),
                        mult,
                    )
                    nc.tensor.matmul(
                        tAB[0:1, 0:512], ones_col[:], WP[:, 0:4, :],
                        start=(i == 0), stop=(i == NT - 1),
                    )
                    nc.tensor.matmul(
                        tAB[0:1, 512:896], ones_col[:], WP[:, 4:7, :],
                        start=(i == 0), stop=(i == NT - 1),
                    )

                    # S and expR share one tile so a single PE colsum
                    # accumulates both lig_score and denR
                    SR = kp.tile([128, 2, F], bf16, tag=f"sr_{s}_{i}", name=f"sr_{s}_{i}")
                    S = SR[:, 0, :]
                    nc.vector.tensor_mul(S, num, rec_b[:])

                    psc = sp.tile([128, 1], f32, tag="psc")
                    nc.vector.tensor_reduce(psc[:], S, axis=mybir.AxisListType.X, op=add)
                    pscb = kp.tile([128, 1], bf16, tag=f"pscb_{s}_{i}", name=f"pscb_{s}_{i}")
                    nc.vector.tensor_copy(pscb[:], psc[:])
                    pscb_tiles[i] = pscb

                    STp = pss.tile([128, 128], bf16, tag="LCT", name=f"STp_{s}_{i}", bufs=2)
                    nc.tensor.transpose(STp[:], S, id_t[:])

                    EFT = sp.tile([128, 128], bf16, tag="EFT")
                    nc.scalar.activation(EFT[:], STp[:], Exp, bias=fmn_col[:])
                    denF = pss.tile([1, 128], f32, tag="LCT", name=f"denF_{s}_{i}", bufs=2)
                    nc.tensor.matmul(denF[:], ones_col[:], EFT[:], start=True, stop=True)
                    recF = sp.tile([1, 128], f32, tag="recF")
                    nc.vector.reciprocal_approx_fast(recF[:], denF[:])
                    recFb = sp.tile([1, 128], bf16, tag="recFb")
                    nc.vector.tensor_copy(recFb[:], recF[:])
                    recFB = sp.tile([128, 128], bf16, tag="recFB")
                    nc.gpsimd.partition_broadcast(recFB[:], recFb[:])
                    attnFT = sp.tile([128, 128], bf16, tag="attnFT")
                    nc.vector.tensor_mul(attnFT[:], EFT[:], recFB[:])

                    LCT = pss.tile([128, 128], f32, tag="LCT", bufs=2)
                    nc.tensor.matmul(LCT[:], lig_t[:], attnFT[:], start=True, stop=True)
                    LCTs = sp.tile([128, 128], bf16, tag="LCTs")
                    nc.scalar.copy(LCTs[:], LCT[:])
                    PAdd = pss.tile([128, 128], f32, tag="PAdd", bufs=2)
                    nc.tensor.matmul(PAdd[:], LCTs[:], wr_t[:], start=True, stop=True)

                    prEn = kp.tile([128, D], bf16, tag=f"pren_{s}_{i}", name=f"pren_{s}_{i}")
                    nc.vector.tensor_add(prEn[:], PAdd[:], Pemb)
                    nc.vector.tensor_add(prEn[:], prEn[:], brB[:])
                    prEn_tiles[i] = prEn

                    expR = SR[:, 1, :]
                    nc.scalar.activation(expR, S, Exp, bias=pmn_cols[:, i:i + 1])
                    expR_tiles[i] = expR
                    nc.tensor.matmul(
                        lsdr[0:1, :], ones_col[:], SR[:],
                        start=(i == 0), stop=(i == NT - 1),
                    )

                    nc.tensor.matmul(
                        ppoolU[0:1, :], pscb[:], prEn[:],
                        start=(i == 0), stop=(i == NT - 1),
                    )

                # ---------------- phase B: per sample ----------------
                recR = sm.tile([1, 128], f32, tag="recR")
                nc.vector.reciprocal_approx_fast(recR[:], lsdr[0:1, 1, :])
                recRTp = pss.tile([128, 1], f32, tag="PAdd", name=f"recRTp_{s}", bufs=2)
                nc.tensor.transpose(recRTp[:], recR[:], id1f[:])
                recRcol = sm.tile([128, 1], f32, tag="recRcol")
                nc.vector.tensor_copy(recRcol[:], recRTp[:])

                PCT = pss.tile([128, 128], f32, tag="LCT", name="PCT", bufs=2)
                for i in range(NT):
                    nc.tensor.matmul(
                        PCT[:], pemb_tiles[i], expR_tiles[i],
                        start=(i == 0), stop=(i == NT - 1),
                    )
                PCTs = sm.tile([128, 128], bf16, tag="PCTs")
                nc.scalar.copy(PCTs[:], PCT[:])
                LAdd = pss.tile([128, 128], f32, tag="PAdd", name="LAdd", bufs=2)
                nc.tensor.matmul(LAdd[:], PCTs[:], wf_t[:], start=True, stop=True)
                ligEn = sm.tile([F, D], bf16, tag="ligEn")
                nc.vector.tensor_scalar_mul(ligEn[:], LAdd[:], recRcol[:])
                nc.vector.tensor_add(ligEn[:], ligEn[:], lig_t[:])
                nc.vector.tensor_add(ligEn[:], ligEn[:], bfB[:])

                # lig score row -> column
                lsc = sm.tile([1, 128], bf16, tag="lsc")
                nc.vector.tensor_copy(lsc[:], lsdr[0:1, 0, :])
                lscT = pss.tile([128, 1], bf16, tag="PAdd", name="lscT", bufs=2)
                nc.tensor.transpose(lscT[:], lsc[:], id_t[0:1, 0:1])
                lsc_col = sm.tile([128, 1], bf16, tag="lsccol")
                nc.vector.tensor_copy(lsc_col[:], lscT[:])

                lpoolU = pss.tile([1, 128], f32, tag="LCT", name="lpoolU", bufs=2)
                nc.tensor.matmul(lpoolU[:], lsc_col[:], ligEn[:], start=True, stop=True)

                # totals
                psall = sm.tile([128, NT], f32, tag="psall")
                for i in range(NT):
                    nc.vector.tensor_copy(psall[:, i:i + 1], pscb_tiles[i][:])
                psum_col = sm.tile([128, 1], f32, tag="psumcol")
                nc.vector.tensor_reduce(psum_col[:], psall[:], axis=mybir.AxisListType.X, op=add)
                psum_colb = sm.tile([128, 1], bf16, tag="psumcolb")
                nc.vector.tensor_copy(psum_colb[:], psum_col[:])
                totPp = pss.tile([1, 1], f32, tag="PAdd", name="totPp", bufs=2)
                nc.tensor.matmul(totPp[:], psum_colb[:], ones_col[:], start=True, stop=True)
                totP = sm.tile([1, 1], f32, tag="totP")
                nc.vector.tensor_scalar_add(totP[:], totPp[:], 1e-8)
                totPr = sm.tile([1, 1], f32, tag="totPr")
                nc.vector.reciprocal_approx_fast(totPr[:], totP[:])

                totL = sm.tile([1, 1], f32, tag="totL")
                nc.vector.tensor_reduce(totL[:], lsc[:], axis=mybir.AxisListType.X, op=add)
                nc.vector.tensor_scalar_add(totL[:], totL[:], 1e-8)
                totLr = sm.tile([1, 1], f32, tag="totLr")
                nc.vector.reciprocal_approx_fast(totLr[:], totL[:])

                # feature row for this sample (partition 0)
                feat_s = sm.tile([1, FEAT], f32, tag="featF", name=f"featF{s}", bufs=2)
                nc.vector.tensor_mul(
                    feat_s[:, 0:D], ppoolU[0:1, :], totPr.broadcast_to([1, 128])
                )
                nc.vector.tensor_mul(
                    feat_s[:, D:2 * D], lpoolU[:], totLr.broadcast_to([1, 128])
                )
                tk = sm.tile([1, K], f32, tag="tk", name=f"tk{s}", bufs=2)
                nc.vector.tensor_reduce(
                    tk[:], tAB.rearrange("p (k f) -> p k f", k=K),
                    axis=mybir.AxisListType.X, op=add,
                )
                nc.vector.tensor_copy(feat_s[:, 2 * D:FEAT], tk[:])

                # squared norm now; the Ln/Exp(-0.5) tail is deferred past
                # the MLP so the ACT func table is switched only once
                sq = sm.tile([1, FEAT], f32, tag="sq")
                ss = sm.tile([1, 1], f32, tag="ss", name=f"ss{s}", bufs=2)
                nc.scalar.activation(sq[:], feat_s[:], Square, accum_out=ss[:])
                norm_defer.append((s, feat_s, ss))

                # transposed feature chunks for the MLP (column s)
                for c in range(3):
                    lo, hi = c * 128, min((c + 1) * 128, FEAT)
                    ftp = pss.tile([128, BC], f32, tag="PAdd", name=f"ftp{s}_{c}", bufs=2)
                    nc.tensor.transpose(
                        ftp[0:hi - lo, 0:1], feat_s[:, lo:hi], id1f[:]
                    )
                    nc.vector.tensor_copy(featT[c][0:hi - lo, s:s + 1], ftp[0:hi - lo, 0:1])

            # ---------------- MLP head (both samples) ----------------
            h1 = []
            for j in range(4):
                h1ps = pss.tile([128, BC], f32, tag="LCT", name=f"h1ps{j}", bufs=2)
                for c in range(3):
                    nc.tensor.matmul(
                        h1ps[:], w1_t[c][:, j * 128:(j + 1) * 128], featT[c][:],
                        start=(c == 0), stop=(c == 2),
                    )
                h1j = sm.tile([128, BC], f32, tag=f"h1_{j}", name=f"h1_{j}", bufs=1)
                nc.scalar.activation(h1j[:], h1ps[:], Relu, bias=b1_t[:, j:j + 1])
                h1.append(h1j)

            h2 = []
            for j2 in range(2):
                h2ps = pss.tile([128, BC], f32, tag="PAdd", name=f"h2ps{j2}", bufs=2)
                for j in range(4):
                    nc.tensor.matmul(
                        h2ps[:], w2_t[j][:, j2 * 128:(j2 + 1) * 128], h1[j][:],
                        start=(j == 0), stop=(j == 3),
                    )
                h2j = sm.tile([128, BC], f32, tag=f"h2_{j2}", name=f"h2_{j2}", bufs=1)
                last_relu = nc.scalar.activation(h2j[:], h2ps[:], Relu, bias=b2_t[:, j2:j2 + 1])
                h2.append(h2j)

            pps = pss.tile([1, BC], f32, tag="PAdd", name="pps", bufs=2)
            for j2 in range(2):
                nc.tensor.matmul(
                    pps[:], w3_t[j2][:], h2[j2][:], start=(j2 == 0), stop=(j2 == 1)
                )
            prow = sm.tile([1, BC], f32, tag="prow")
            nc.vector.tensor_scalar_add(prow[:], pps[:], float(b3))
            nc.sync.dma_start(out=predT[:], in_=prow[:])

            for s, feat_s, ss in norm_defer:
                lnss = sm.tile([1, 1], f32, tag="lnss")
                ln_i = nc.scalar.activation(lnss[:], ss[:], Ln)
                tile.add_dep_helper(
                    ln_i.ins, last_relu.ins, sync=False,
                    reason="group Ln after exp-table ACT ops")
                rsn = sm.tile([1, 1], f32, tag="rsn")
                nc.scalar.activation(rsn[:], lnss[:], Exp, scale=-0.5)
                nc.vector.tensor_scalar_min(rsn[:], rsn[:], 1e12)
                fnorm = sm.tile([1, FEAT], f32, tag="fnorm")
                nc.vector.tensor_scalar_mul(fnorm[:], feat_s[:], rsn[:])
                nc.sync.dma_start(out=featn[s:s + 1, :], in_=fnorm[:])

    nc.compile()
    return nc


def _prep_inputs(ligand_emb, protein_emb, logits, fg_mask, prot_mask,
                 type_weight, Wr, br, Wf, bf, W1, b1, W2, b2, W3, b3):
    """Host-side prep: dtype/layout transforms + per-core sharding."""
    w = _softplus64(type_weight).astype(np.float64)

    lgKF = np.empty((B, R, K + 1, F), BF16)            # [B,R,8,128]
    lgKF[:, :, 0:K, :] = np.transpose(
        np.asarray(logits, np.float32), (0, 1, 3, 2)).astype(BF16)
    lgKF[:, :, K, :] = np.asarray(protein_emb, np.float32).astype(BF16)
    lebf = np.asarray(ligand_emb, np.float32).astype(BF16)
    fm = np.asarray(fg_mask, np.float32)
    pm = np.asarray(prot_mask, np.float32)
    fmneg = ((1.0 - fm) * NEG).astype(np.float32)
    pmneg = ((1.0 - pm) * NEG).astype(np.float32)
    # pm as [B, 128, NT] columns
    pmT = np.ascontiguousarray(pm.reshape(B, NT, 128).transpose(0, 2, 1))
    pmnT = np.ascontiguousarray(pmneg.reshape(B, NT, 128).transpose(0, 2, 1))

    wrT = np.asarray(Wr, np.float32).T.astype(BF16).copy()
    wfT = np.asarray(Wf, np.float32).T.astype(BF16).copy()
    brb = np.broadcast_to(
        np.asarray(br, np.float32).reshape(1, D), (128, D)).astype(BF16).copy()
    bfb = np.broadcast_to(
        np.asarray(bf, np.float32).reshape(1, D), (128, D)).astype(BF16).copy()
    fmBrep = np.broadcast_to(
        np.asarray(fg_mask, np.float32)[:, None, :], (B, 128, F)
    ).astype(BF16).copy()

    w1T = np.zeros((384, H), np.float32)
    w1T[:FEAT] = np.asarray(W1, np.float32).T
    w1d = np.ascontiguousarray(w1T.reshape(3, 128, H))
    b1d = np.ascontiguousarray(
        np.asarray(b1, np.float32).reshape(4, 128).T
    )
    w2d = np.ascontiguousarray(np.asarray(W2, np.float32).T.reshape(4, 128, H // 2))
    b2d = np.ascontiguousarray(
        np.asarray(b2, np.float32).reshape(2, 128).T
    )
    w3d = np.ascontiguousarray(np.asarray(W3, np.float32).T.reshape(2, 128, 1))
    identity = np.eye(128, dtype=np.float32).astype(BF16)

    wrd = np.broadcast_to(
        np.asarray(w, np.float32)[None, :, None], (128, K, F)
    ).astype(BF16).copy()
    shared = dict(
        wrd=wrd, wrT=wrT, wfT=wfT, brd=brb, bfd=bfb,
        w1d=w1d, b1d=b1d, w2d=w2d, b2d=b2d, w3d=w3d, ident=identity,
    )
    in_maps = []
    for c in range(NCORES):
        sl = slice(c * BC, (c + 1) * BC)
        m = dict(shared)
        m.update(
            lg=np.ascontiguousarray(lgKF[sl]),
            le=np.ascontiguousarray(lebf[sl]),
            fmr=np.ascontiguousarray(fm[sl]),
            fmnc=np.ascontiguousarray(fmneg[sl]),
            pmT=np.ascontiguousarray(pmT[sl]),
            pmnT=np.ascontiguousarray(pmnT[sl]),
            fmBd=np.ascontiguousarray(fmBrep[sl]),
        )
        in_maps.append(m)
    return w, float(np.asarray(b3).reshape(-1)[0]), in_maps


_CACHED = {}


def kernel(**inputs):
    w, b3v, in_maps = _prep_inputs(**inputs)

    key = (tuple(np.round(w, 10)), round(b3v, 10))
    if key not in _CACHED:
        _CACHED[key] = build_program(w, b3v)
    nc = _CACHED[key]

    from concourse.bass_utils import run_bass_kernel_spmd
    res = run_bass_kernel_spmd(nc, in_maps, list(range(NCORES))).results

    pred = np.zeros((B, 1), np.float32)
    featn = np.zeros((B, FEAT), np.float32)
    for c in range(NCORES):
        pred[c * BC:(c + 1) * BC, 0] = np.asarray(res[c]["predT"], np.float32)[0]
        featn[c * BC:(c + 1) * BC] = np.asarray(res[c]["featn"], np.float32)
    return pred, featn
